# revision 5
# baseline (speedup 1.0000x reference)
"""GQA attention (B=2, T=2048, C=2048, 32 Q heads, 8 KV heads, causal) on 8 TRN2 cores.

Sharding: core c -> batch b=c//4, kv-head pair g=c%4 (kv heads 2g,2g+1; q heads
8g..8g+7).  Out-proj is column-sharded (the 512 columns belonging to this
core's q heads); partial outputs are summed with a ReduceScatter over each
batch's 4-core group, so core c returns a [512, 2048] slice of out^T (dout rows
512*(c%4)..512*(c%4+1) for batch c//4).

On-device layout is feature-major ("transposed"): the host feeds x^T and
pre-transposed weight shards so every matmul contracts over the partition dim
with zero on-device transposes (except V, transposed once on the PE so the
softmax denominator can ride along the att@V matmul as an appended ones
column).
"""

import numpy as np

import concourse.bass as bass  # noqa: F401  (bass types used via nc handles)
import concourse.mybir as mybir
import concourse.tile as tile
from concourse import bacc
from concourse.bass_utils import run_bass_kernel_spmd
from concourse.masks import make_identity

F32 = mybir.dt.float32
B, T, C = 2, 2048, 2048
HQ, HKV, D = 32, 8, 64
G = HQ // HKV  # 4 q heads per kv head
N_CORES = 8
P = 128
TQ = 512  # t-chunk (matmul moving free dim)
N_TQ = T // TQ  # 4
N_TT = T // P  # 16 (c-tiles and t-tiles)
SCALE = 1.0 / 8.0  # 1/sqrt(D)
# per-core shards
DQ = HQ * D // 4  # 512 q dims per core
DKV = 2 * D  # 128 kv dims per core (2 kv heads)
H_LOC = 8  # local q heads
HEAD_PERM = [0, 4, 1, 5, 2, 6, 3, 7]  # head h -> dtile h%4, partition (h//4)*64

_NC_CACHE = {}


def _build_nc():
    if "nc" in _NC_CACHE:
        return _NC_CACHE["nc"]
    nc = bacc.Bacc("TRN2", target_bir_lowering=False, debug=False, num_devices=N_CORES)

    xt_e = nc.dram_tensor("xt", [C, T], F32, kind="ExternalInput")
    wqt_e = nc.dram_tensor("wqt", [C, DQ], F32, kind="ExternalInput")
    wkt_e = nc.dram_tensor("wkt", [C, DKV], F32, kind="ExternalInput")
    wvt_e = nc.dram_tensor("wvt", [C, DKV], F32, kind="ExternalInput")
    wot_e = nc.dram_tensor("wot", [DQ, C], F32, kind="ExternalInput")
    out_e = nc.dram_tensor("out", [512, T], F32, kind="ExternalOutput")

    with tile.TileContext(nc) as tc:
        with (
            tc.tile_pool(name="pers", bufs=1) as pers,
            tc.tile_pool(name="psum", bufs=1, space="PSUM") as pp,
            tc.tile_pool(name="dram", bufs=1, space="DRAM") as dp,
        ):
            # ---- persistent SBUF tensors --------------------------------
            s_qt = pers.tile([P, 4 * T], F32, tag="qt")  # Q^T: dtile j cols j*T..
            s_kt = pers.tile([P, T], F32, tag="kt")  # K^T: kv-local e=128 partitions
            s_va = pers.tile([P, N_TT * 130], F32, tag="va")  # V-nat+ones per (tt,kvl)
            s_ot = pers.tile([P, 4 * T], F32, tag="ot")  # attn out^T (same layout as qt)
            s_wo = pers.tile([P, 4 * T], F32, tag="wo")  # wo^T: 4 dh-tiles x [128, 2048]
            s_mask = pers.tile([P, 4 * TQ], F32, tag="mask")  # 4 diag masks [128,512]
            s_ident = pers.tile([P, P], F32, tag="ident")

            nc.sync.dma_start(
                out=s_wo[:].rearrange("p (k d) -> p k d", k=4),
                in_=wot_e[:, :].rearrange("(k p) d -> p k d", p=P),
            )
            make_identity(nc, s_ident[:])
            # diag masks: mask[oi][i, j] = 1 if i <= j - 128*oi else 0
            nc.gpsimd.memset(s_mask[:], 0.0)
            for oi in range(4):
                nc.gpsimd.affine_select(
                    out=s_mask[:, oi * TQ : (oi + 1) * TQ],
                    in_=s_mask[:, oi * TQ : (oi + 1) * TQ],
                    compare_op=mybir.AluOpType.is_gt,
                    fill=1.0,
                    base=oi * P,
                    channel_multiplier=1,
                    pattern=[[-1, TQ]],
                )
            # ones column for the softmax-denominator trick (col 64 of each 65-block)
            nc.gpsimd.memset(s_va[:], 1.0)

            # ---- phase 1: projections (Q^T, K^T, V^T) -------------------
            with tc.tile_pool(name="ph1", bufs=1) as ph1, tc.tile_pool(name="xp", bufs=3) as xp:
                s_wq = ph1.tile([P, N_TT * DQ], F32, tag="wq")
                s_wk = ph1.tile([P, N_TT * DKV], F32, tag="wk")
                s_wv = ph1.tile([P, N_TT * DKV], F32, tag="wv")
                s_vt = ph1.tile([P, T], F32, tag="vt")
                nc.sync.dma_start(
                    out=s_wq[:].rearrange("p (k d) -> p k d", k=N_TT),
                    in_=wqt_e[:, :].rearrange("(k p) d -> p k d", p=P),
                )
                nc.sync.dma_start(
                    out=s_wk[:].rearrange("p (k d) -> p k d", k=N_TT),
                    in_=wkt_e[:, :].rearrange("(k p) d -> p k d", p=P),
                )
                nc.sync.dma_start(
                    out=s_wv[:].rearrange("p (k d) -> p k d", k=N_TT),
                    in_=wvt_e[:, :].rearrange("(k p) d -> p k d", p=P),
                )

                for tq in range(N_TQ):
                    psq = [pp.tile([P, TQ], F32, tag="mm", bufs=6, name=f"psq{_j}") for _j in range(4)]
                    psk = pp.tile([P, TQ], F32, tag="mm", bufs=6, name="psk")
                    psv = pp.tile([P, TQ], F32, tag="mm", bufs=6, name="psv")
                    for kg in range(N_TT // 2):  # DMA 2 c-tiles at a time
                        xt_t = xp.tile([P, 2, TQ], F32, tag="xt", name="xt_t")
                        nc.sync.dma_start(
                            out=xt_t[:],
                            in_=xt_e[
                                kg * 2 * P : (kg + 1) * 2 * P, tq * TQ : (tq + 1) * TQ
                            ].rearrange("(k p) t -> p k t", p=P),
                        )
                        for kk in range(2):
                            k = kg * 2 + kk
                            st, sp = k == 0, k == N_TT - 1
                            rhs = xt_t[:, kk, :]
                            for j in range(4):
                                nc.tensor.matmul(
                                    psq[j][:],
                                    s_wq[:, k * DQ + j * P : k * DQ + (j + 1) * P],
                                    rhs,
                                    start=st,
                                    stop=sp,
                                )
                            nc.tensor.matmul(
                                psk[:],
                                s_wk[:, k * DKV : (k + 1) * DKV],
                                rhs,
                                start=st,
                                stop=sp,
                            )
                            nc.tensor.matmul(
                                psv[:],
                                s_wv[:, k * DKV : (k + 1) * DKV],
                                rhs,
                                start=st,
                                stop=sp,
                            )
                    for j in range(4):
                        nc.vector.tensor_copy(
                            s_qt[:, j * T + tq * TQ : j * T + (tq + 1) * TQ], psq[j][:]
                        )
                    nc.vector.tensor_copy(s_kt[:, tq * TQ : (tq + 1) * TQ], psk[:])
                    nc.vector.tensor_copy(s_vt[:, tq * TQ : (tq + 1) * TQ], psv[:])

                # V^T -> V natural (PE transpose), into 65-wide aug blocks
                for tt in range(N_TT):
                    pst = pp.tile([P, P], F32, tag="mm", bufs=6, name="pst")
                    nc.tensor.transpose(
                        pst[:], s_vt[:, tt * P : (tt + 1) * P], s_ident[:]
                    )
                    nc.vector.tensor_copy(
                        s_va[:, tt * 130 : tt * 130 + 64], pst[:, 0:64]
                    )
                    nc.vector.tensor_copy(
                        s_va[:, tt * 130 + 65 : tt * 130 + 129], pst[:, 64:128]
                    )

            # ---- phase 2: attention; phase 3: out-proj + RS, per t-chunk -
            with (
                tc.tile_pool(name="ep", bufs=3) as ep,
                tc.tile_pool(name="np_", bufs=2) as npo,
            ):
                for qc in range(N_TQ):
                    n_tk = 4 * (qc + 1)
                    for h in range(H_LOC):
                        # host permutes heads to [0,4,1,5,2,6,3,7]: head h sits
                        # in dtile h%4 at partition (h//4)*64 == its kv base
                        kvl, j, poff = h // G, h % 4, (h // G) * 64
                        pso = pp.tile([65, TQ], F32, tag="attv", bufs=2, name="pso")
                        qs = s_qt[poff : poff + 64, j * T + qc * TQ : j * T + (qc + 1) * TQ]
                        for tk in range(n_tk):
                            pss = pp.tile([P, TQ], F32, tag="mm", bufs=6, name="pss")
                            nc.tensor.matmul(
                                pss[:],
                                s_kt[kvl * 64 : kvl * 64 + 64, tk * P : (tk + 1) * P],
                                qs,
                                start=True,
                                stop=True,
                            )
                            et = ep.tile([P, TQ], F32, tag="et", name="et")
                            nc.scalar.activation(
                                out=et[:],
                                in_=pss[:],
                                func=mybir.ActivationFunctionType.Exp,
                                scale=SCALE,
                            )
                            oi = tk - qc * 4
                            if oi >= 0:  # diagonal-spanning tile: causal mask
                                nc.vector.tensor_mul(
                                    et[:], et[:], s_mask[:, oi * TQ : (oi + 1) * TQ]
                                )
                            nc.tensor.matmul(
                                pso[:],
                                s_va[:, tk * 130 + kvl * 65 : tk * 130 + kvl * 65 + 65],
                                et[:],
                                start=(tk == 0),
                                stop=(tk == n_tk - 1),
                            )
                        rc = npo.tile([1, TQ], F32, tag="rc", name="rc")
                        nc.vector.reciprocal(rc[:], pso[64:65, :])
                        rb = npo.tile([64, TQ], F32, tag="rb", name="rb")
                        nc.gpsimd.partition_broadcast(rb[:], rc[:])
                        nc.vector.tensor_mul(
                            s_ot[poff : poff + 64, j * T + qc * TQ : j * T + (qc + 1) * TQ],
                            pso[0:64, :],
                            rb[:],
                        )

                    # out-proj for this t-chunk + ReduceScatter of the partial
                    yb = dp.tile([C, TQ], F32, tag="yb", bufs=2, name="yb")
                    for m in range(N_TT):
                        psy = pp.tile([P, TQ], F32, tag="mm", bufs=6, name="psy")
                        for j in range(4):
                            nc.tensor.matmul(
                                psy[:],
                                s_wo[:, j * T + m * P : j * T + (m + 1) * P],
                                s_ot[:, j * T + qc * TQ : j * T + (qc + 1) * TQ],
                                start=(j == 0),
                                stop=(j == 3),
                            )
                        sy = ep.tile([P, TQ], F32, tag="sy", name="sy", bufs=3)
                        nc.vector.tensor_copy(sy[:], psy[:])
                        nc.sync.dma_start(out=yb[m * P : (m + 1) * P, :], in_=sy[:])
                    rs = dp.tile([512, TQ], F32, tag="rs", bufs=2, name="rs")
                    nc.gpsimd.collective_compute(
                        "ReduceScatter",
                        mybir.AluOpType.add,
                        replica_groups=[[0, 1, 2, 3], [4, 5, 6, 7]],
                        ins=[yb[:].opt()],
                        outs=[rs[:].opt()],
                    )
                    nc.sync.dma_start(
                        out=out_e[:, qc * TQ : (qc + 1) * TQ], in_=rs[:]
                    )

    nc.compile()
    _NC_CACHE["nc"] = nc
    return nc


def kernel(x, wq, wk, wv, wo):
    x = np.asarray(x, dtype=np.float32)
    wq = np.asarray(wq, dtype=np.float32)
    wk = np.asarray(wk, dtype=np.float32)
    wv = np.asarray(wv, dtype=np.float32)
    wo = np.asarray(wo, dtype=np.float32)

    nc = _build_nc()
    in_maps = []
    for c in range(N_CORES):
        b, g = c // 4, c % 4
        in_maps.append(
            {
                "xt": np.ascontiguousarray(x[b].T),
                "wqt": np.ascontiguousarray(
                    wq[DQ * g : DQ * (g + 1), :]
                    .reshape(8, D, C)[HEAD_PERM]
                    .reshape(DQ, C)
                    .T
                ),
                "wkt": np.ascontiguousarray(wk[DKV * g : DKV * (g + 1), :].T),
                "wvt": np.ascontiguousarray(wv[DKV * g : DKV * (g + 1), :].T),
                "wot": np.ascontiguousarray(
                    wo[:, DQ * g : DQ * (g + 1)]
                    .reshape(C, 8, D)[:, HEAD_PERM]
                    .reshape(C, DQ)
                    .T
                ),
            }
        )
    res = run_bass_kernel_spmd(nc, in_maps, list(range(N_CORES)))
    out = np.empty((B, T, C), dtype=np.float32)
    for c in range(N_CORES):
        b, r = c // 4, c % 4
        out[b, :, 512 * r : 512 * (r + 1)] = res.results[c]["out"].T
    return out
